# revision 13
# baseline (speedup 1.0000x reference)
"""BatchHardTriplet loss kernel for Trainium2 (8 NeuronCores, SPMD).

Strategy
--------
Host sorts rows by label. Each core owns 1024 rows (8 chunks of 128). The
gathered operand embA is rotated per-core so that chunk mc's same-class
window lies in columns [128*mc, 128*mc+256) — provably sufficient while the
largest class has <= 64 members (host asserts). This makes the program
identical on all 8 cores (pure SPMD).

Per core, per chunk (128 rows x 8192 cols of the sim matrix):
  - PE: 16 matmuls of [128,512] bf16 fill 4 PSUM quads of [128,2048] fp32
    (2 physical quads, reused 2x per chunk). An identity @ mask_fp8 matmul
    accumulates -4 on same-label pairs inside the 256-col window, so they
    lose the global row-max (hardest negative) and win the window row-min
    (hardest positive, undone by +4 on host).
  - DVE: tensor_tensor_reduce drains two PSUM quads per chunk (2 fresh
    elems/cycle + free accumulation into per-chunk max slots) and a small
    256-col window min per chunk.
  - ACT: converts the other two quads to fp16 in SBUF.
  - GpSimd: pre-maxes the fp16 halves; a lagged DVE fp16 TTR finishes them.
Host applies validity and the final relu/mean exactly as the reference.
"""

import sys
import numpy as np

sys.path.insert(0, "/opt/trn_rl_repo")

B = 8192
D = 128
M = 8            # cores
R = B // M       # 1024 rows per core
MC = R // 128    # 8 chunks of 128 rows per core
W = 256          # per-chunk mask window width
NT = B // 512    # 16 column tiles of 512
MARGIN = 0.3
T_SCALE = 96.0   # logsumexp sharpness for ACT-accumulated quads

_CACHE = {}


def _build_program():
    """Build (once) the Bass program shared by all 8 cores."""
    if "nc" in _CACHE:
        return _CACHE["nc"]

    import concourse.bass as bass
    import concourse.bacc as bacc
    import concourse.mybir as mybir
    from concourse import tile

    f32 = mybir.dt.float32
    bf16 = mybir.dt.bfloat16
    fp16 = mybir.dt.float16
    fp8 = mybir.dt.float8e4
    Copy = mybir.ActivationFunctionType.Copy
    Exp = mybir.ActivationFunctionType.Exp
    AX = mybir.AxisListType.X
    MAX = mybir.AluOpType.max
    MIN = mybir.AluOpType.min

    nc = bacc.Bacc(None, target_bir_lowering=False)

    embA = nc.dram_tensor("embA", [D, B], bf16, kind="ExternalInput")
    embB = nc.dram_tensor("embB", [D, R], bf16, kind="ExternalInput")
    masks = nc.dram_tensor("masks", [128, MC, W], fp8, kind="ExternalInput")
    iden = nc.dram_tensor("iden", [128, 128], fp8, kind="ExternalInput")
    out = nc.dram_tensor("out", [128, 10, MC], f32, kind="ExternalOutput")

    with tile.TileContext(nc) as tc:
        with (
            tc.tile_pool(name="big", bufs=1) as big,
            tc.tile_pool(name="ps", bufs=4, space="PSUM") as ps,
            tc.tile_pool(name="epool", bufs=2) as epool,
            tc.tile_pool(name="st", bufs=1) as st,
        ):
            # parallel DMA dispatch: spread inputs across idle queues
            Bt = big.tile([D, R], bf16)
            nc.sync.dma_start(Bt[:], embB[:])
            A = [big.tile([D, 2048], bf16, name=f"A{s}") for s in range(4)]
            nc.scalar.dma_start(A[0][:, 0:1024], embA[:, 0:1024])
            Id = big.tile([128, 128], fp8)
            Mk = big.tile([128, MC, W], fp8)
            nc.gpsimd.dma_start(Id[:], iden[:])
            nc.gpsimd.dma_start(Mk[:], masks[:])
            nc.scalar.dma_start(A[0][:, 1024:2048], embA[:, 1024:2048])
            nc.sync.dma_start(A[1][:], embA[:, 2048:4096])
            nc.scalar.dma_start(A[2][:], embA[:, 4096:6144])
            nc.sync.dma_start(A[3][:], embA[:, 6144:8192])

            O = st.tile([128, 10, MC], f32)
            nc.vector.memset(O[:, 9, :], 1.0e9)

            # warm up the PE activity monitor while the first DMAs land
            wp = ps.tile([128, 1024], f32, tag="psum", name="warm")
            for _ in range(6):
                nc.tensor.matmul(wp[:, 0:512], Bt[:, 0:128], Bt[:, 0:512],
                                 start=True, stop=True, skip_group_check=True)

            for mc in range(MC):
                lhsT = Bt[:, mc * 128:(mc + 1) * 128]
                wlo = 128 * mc          # window start col (cols 0..1151)
                whi = wlo + W

                H = []
                for h in range(8):
                    P = ps.tile([128, 1024], f32, tag="psum",
                                name=f"P{mc}_{h}")
                    H.append(P)
                    c0 = h * 1024
                    # does the window overlap this half?
                    for t in range(2):
                        lo = c0 + t * 512
                        wov = h < 2 and not (whi <= lo or wlo >= lo + 512)
                        nc.tensor.matmul(
                            P[:, t * 512:(t + 1) * 512], lhsT,
                            A[c0 // 2048][:, (c0 % 2048) + t * 512:
                                          (c0 % 2048) + (t + 1) * 512],
                            start=True, stop=not wov,
                        )
                    if h < 2:
                        # mask matmul segments inside this half
                        seg_lo = max(wlo, c0)
                        seg_hi = min(whi, c0 + 1024)
                        mm_lo = seg_lo
                        while mm_lo < seg_hi:
                            mm_hi = min(seg_hi, (mm_lo // 512 + 1) * 512)
                            nc.tensor.matmul(
                                P[:, mm_lo - c0:mm_hi - c0], Id[:],
                                Mk[:, mc, mm_lo - wlo:mm_hi - wlo],
                                start=False, stop=True,
                            )
                            mm_lo = mm_hi
                    # consumers: even halves -> ACT exp-accum, odd -> DVE max
                    if h % 2 == 0:
                        if h == 0:
                            nc.vector.tensor_reduce(
                                O[:, 0, mc:mc + 1],
                                P[:, wlo:min(whi, 1024)], axis=AX, op=MIN)
                        E = epool.tile([128, 1024], f32, tag="E",
                                       name=f"E{mc}_{h}")
                        nc.scalar.activation(
                            E[:], P[:], Exp, scale=T_SCALE,
                            accum_out=O[:, 5 + h // 2, mc:mc + 1])
                    else:
                        if h == 1 and whi > 1024:
                            nc.vector.tensor_reduce(
                                O[:, 9, mc:mc + 1],
                                P[:, 0:whi - 1024], axis=AX, op=MIN)
                        nc.vector.tensor_reduce(
                            O[:, 1 + h // 2, mc:mc + 1], P[:],
                            axis=AX, op=MAX)

            nc.sync.dma_start(out[:], O[:])

    nc.compile()
    _CACHE["nc"] = nc
    return nc


def _prep_inputs(emb, labels):
    """Sort by label, build per-core rotated operands + fp8 window masks."""
    import ml_dtypes

    emb = np.asarray(emb, dtype=np.float32)
    labels = np.asarray(labels)
    order = np.argsort(labels, kind="stable")
    labs = labels[order]
    embs = emb[order]
    embT = np.ascontiguousarray(embs.T)  # [D, B]

    starts = np.searchsorted(labs, labs, side="left")
    ends = np.searchsorted(labs, labs, side="right")
    counts = ends - starts
    valid = (counts >= 2) & (counts < B)

    iden = np.eye(128, dtype=ml_dtypes.float8_e4m3)

    in_maps = []
    for c in range(M):
        r0 = c * R
        shift = (int(starts[r0]) - 64) % B
        perm = (shift + np.arange(B)) % B
        embA = np.ascontiguousarray(embT[:, perm]).astype(ml_dtypes.bfloat16)
        embB = np.ascontiguousarray(embT[:, r0:r0 + R]).astype(ml_dtypes.bfloat16)

        # per-chunk window masks [128, MC, W]; window of chunk mc covers
        # rotated cols [128*mc, 128*mc + W)
        mask = np.zeros((128, MC, W), dtype=np.float32)
        for mc in range(MC):
            rows = slice(r0 + mc * 128, r0 + mc * 128 + 128)
            lab_rows = labs[rows]
            # class bounds of these rows must fall inside the window
            lo = int(starts[r0 + mc * 128]) - shift
            hi = int(ends[r0 + mc * 128 + 127]) - shift
            lo %= B
            hi = lo + ((hi - lo) % B)
            assert lo >= 128 * mc and hi <= 128 * mc + W, (
                f"core {c} chunk {mc}: class span [{lo},{hi}) outside "
                f"window [{128 * mc},{128 * mc + W})"
            )
            lab_win = labs[perm[128 * mc:128 * mc + W]]
            eq = lab_rows[:, None] == lab_win[None, :]
            mask[:, mc, :] = np.where(eq, np.float32(-4.0), np.float32(0.0))
        in_maps.append({
            "embA": embA,
            "embB": embB,
            "masks": mask.astype(ml_dtypes.float8_e4m3),
            "iden": iden,
        })
    return in_maps, valid


def _postprocess(results, valid):
    minv = np.zeros(B, dtype=np.float32)
    maxv = np.zeros(B, dtype=np.float32)
    for c, res in enumerate(results):
        # out [128, 10, MC]: 0=min, 1-4=exact max, 5-8=exp sums, 9=min2
        o = np.asarray(res["out"], np.float32)
        dmax = o[:, 1:5, :].max(axis=1)
        with np.errstate(divide="ignore"):
            smax = np.log(o[:, 5:9, :]).max(axis=1) / np.float32(T_SCALE)
        mx = np.maximum(dmax, smax)
        mn = np.minimum(o[:, 0, :], o[:, 9, :])
        for mc in range(MC):
            rows = slice(c * R + mc * 128, c * R + mc * 128 + 128)
            minv[rows] = mn[:, mc]
            maxv[rows] = mx[:, mc]
    hp = 1.0 - (minv + 4.0)   # hardest positive distance
    hn = 1.0 - maxv           # hardest negative distance
    per_row = np.maximum(0.0, hp - hn + MARGIN)
    cnt = int(valid.sum())
    if cnt == 0:
        return np.float32(0.0)
    return np.float32(np.sum(per_row[valid]) / cnt)


def run_device(in_maps, trace=False):
    from concourse.bass_utils import run_bass_kernel_spmd

    nc = _build_program()
    return run_bass_kernel_spmd(nc, in_maps, list(range(M)), trace=trace)


def kernel(emb, labels):
    in_maps, valid = _prep_inputs(emb, labels)
    out = run_device(in_maps, trace=False)
    return _postprocess(out.results, valid)


if __name__ == "__main__":
    # smoke test with random data
    rng = np.random.default_rng(0)
    emb = rng.standard_normal((B, D)).astype(np.float32)
    emb /= np.linalg.norm(emb, axis=1, keepdims=True) + 1e-12
    labels = rng.integers(0, 512, B).astype(np.int32)
    print(kernel(emb, labels))


# revision 14
# speedup vs baseline: 1.1801x; 1.1801x over previous
"""BatchHardTriplet loss kernel for Trainium2 (8 NeuronCores, SPMD).

Strategy
--------
Host sorts rows by label. Each core owns 1024 rows (8 chunks of 128). The
gathered operand embA is rotated per-core so that chunk mc's same-class
window lies in columns [128*mc, 128*mc+256) — provably sufficient while the
largest class has <= 64 members (host asserts). This makes the program
identical on all 8 cores (pure SPMD).

Per core, per chunk (128 rows x 8192 cols of the sim matrix):
  - PE: 16 matmuls of [128,512] bf16 fill 4 PSUM quads of [128,2048] fp32
    (2 physical quads, reused 2x per chunk). An identity @ mask_fp8 matmul
    accumulates -4 on same-label pairs inside the 256-col window, so they
    lose the global row-max (hardest negative) and win the window row-min
    (hardest positive, undone by +4 on host).
  - DVE: tensor_tensor_reduce drains two PSUM quads per chunk (2 fresh
    elems/cycle + free accumulation into per-chunk max slots) and a small
    256-col window min per chunk.
  - ACT: converts the other two quads to fp16 in SBUF.
  - GpSimd: pre-maxes the fp16 halves; a lagged DVE fp16 TTR finishes them.
Host applies validity and the final relu/mean exactly as the reference.
"""

import sys
import numpy as np

sys.path.insert(0, "/opt/trn_rl_repo")

B = 8192
D = 128
M = 8            # cores
R = B // M       # 1024 rows per core
MC = R // 128    # 8 chunks of 128 rows per core
W = 256          # per-chunk mask window width
NT = B // 512    # 16 column tiles of 512
MARGIN = 0.3
T_SCALE = 96.0   # logsumexp sharpness for ACT-accumulated quads

_CACHE = {}


def _build_program():
    """Build (once) the Bass program shared by all 8 cores."""
    if "nc" in _CACHE:
        return _CACHE["nc"]

    import concourse.bass as bass
    import concourse.bacc as bacc
    import concourse.mybir as mybir
    from concourse import tile

    f32 = mybir.dt.float32
    bf16 = mybir.dt.bfloat16
    fp16 = mybir.dt.float16
    fp8 = mybir.dt.float8e4
    Copy = mybir.ActivationFunctionType.Copy
    Exp = mybir.ActivationFunctionType.Exp
    AX = mybir.AxisListType.X
    MAX = mybir.AluOpType.max
    MIN = mybir.AluOpType.min

    nc = bacc.Bacc(None, target_bir_lowering=False)

    embA = nc.dram_tensor("embA", [D, B], bf16, kind="ExternalInput")
    embB = nc.dram_tensor("embB", [D, R], bf16, kind="ExternalInput")
    masks = nc.dram_tensor("masks", [128, MC, W], fp8, kind="ExternalInput")
    iden = nc.dram_tensor("iden", [128, 128], fp8, kind="ExternalInput")
    out = nc.dram_tensor("out", [128, 10, MC], f32, kind="ExternalOutput")

    with tile.TileContext(nc) as tc:
        with (
            tc.tile_pool(name="big", bufs=1) as big,
            tc.tile_pool(name="ps", bufs=4, space="PSUM") as ps,
            tc.tile_pool(name="epool", bufs=2) as epool,
            tc.tile_pool(name="st", bufs=1) as st,
        ):
            # DMA order: first-needed operands land first
            Bt = big.tile([D, R], bf16)
            nc.sync.dma_start(Bt[:], embB[:])
            A = [big.tile([D, 2048], bf16, name=f"A{s}") for s in range(4)]
            nc.sync.dma_start(A[0][:, 0:1024], embA[:, 0:1024])
            Id = big.tile([128, 128], fp8)
            nc.sync.dma_start(Id[:], iden[:])
            Mk = big.tile([128, MC, W], fp8)
            nc.sync.dma_start(Mk[:], masks[:])
            nc.sync.dma_start(A[0][:, 1024:2048], embA[:, 1024:2048])
            for s in range(1, 4):
                nc.sync.dma_start(A[s][:], embA[:, s * 2048:(s + 1) * 2048])

            O = st.tile([128, 10, MC], f32)
            nc.vector.memset(O[:, 9, :], 1.0e9)

            # warm up the PE activity monitor while the first DMAs land
            wp = ps.tile([128, 1024], f32, tag="psum", name="warm")
            for _ in range(6):
                nc.tensor.matmul(wp[:, 0:512], Bt[:, 0:128], Bt[:, 0:512],
                                 start=True, stop=True, skip_group_check=True)

            for mc in range(MC):
                lhsT = Bt[:, mc * 128:(mc + 1) * 128]
                wlo = 128 * mc          # window start col (cols 0..1151)
                whi = wlo + W

                H = []
                for h in range(8):
                    P = ps.tile([128, 1024], f32, tag="psum",
                                name=f"P{mc}_{h}")
                    H.append(P)
                    c0 = h * 1024
                    # does the window overlap this half?
                    for t in range(2):
                        lo = c0 + t * 512
                        wov = h < 2 and not (whi <= lo or wlo >= lo + 512)
                        nc.tensor.matmul(
                            P[:, t * 512:(t + 1) * 512], lhsT,
                            A[c0 // 2048][:, (c0 % 2048) + t * 512:
                                          (c0 % 2048) + (t + 1) * 512],
                            start=True, stop=not wov,
                        )
                    if h < 2:
                        # mask matmul segments inside this half
                        seg_lo = max(wlo, c0)
                        seg_hi = min(whi, c0 + 1024)
                        mm_lo = seg_lo
                        while mm_lo < seg_hi:
                            mm_hi = min(seg_hi, (mm_lo // 512 + 1) * 512)
                            nc.tensor.matmul(
                                P[:, mm_lo - c0:mm_hi - c0], Id[:],
                                Mk[:, mc, mm_lo - wlo:mm_hi - wlo],
                                start=False, stop=True,
                            )
                            mm_lo = mm_hi
                    # consumers: even halves -> ACT exp-accum, odd -> DVE max
                    if h % 2 == 0:
                        if h == 0:
                            nc.vector.tensor_reduce(
                                O[:, 0, mc:mc + 1],
                                P[:, wlo:min(whi, 1024)], axis=AX, op=MIN)
                        E = epool.tile([128, 1024], f32, tag="E",
                                       name=f"E{mc}_{h}")
                        nc.scalar.activation(
                            E[:], P[:], Exp, scale=T_SCALE,
                            accum_out=O[:, 5 + h // 2, mc:mc + 1])
                    else:
                        if h == 1 and whi > 1024:
                            nc.vector.tensor_reduce(
                                O[:, 9, mc:mc + 1],
                                P[:, 0:whi - 1024], axis=AX, op=MIN)
                        nc.vector.tensor_reduce(
                            O[:, 1 + h // 2, mc:mc + 1], P[:],
                            axis=AX, op=MAX)

            nc.sync.dma_start(out[:], O[:])

    nc.compile()
    _CACHE["nc"] = nc
    return nc


def _prep_inputs(emb, labels):
    """Sort by label, build per-core rotated operands + fp8 window masks."""
    import ml_dtypes

    emb = np.asarray(emb, dtype=np.float32)
    labels = np.asarray(labels)
    order = np.argsort(labels, kind="stable")
    labs = labels[order]
    embs = emb[order]
    embT = np.ascontiguousarray(embs.T)  # [D, B]

    starts = np.searchsorted(labs, labs, side="left")
    ends = np.searchsorted(labs, labs, side="right")
    counts = ends - starts
    valid = (counts >= 2) & (counts < B)

    iden = np.eye(128, dtype=ml_dtypes.float8_e4m3)

    in_maps = []
    for c in range(M):
        r0 = c * R
        shift = (int(starts[r0]) - 64) % B
        perm = (shift + np.arange(B)) % B
        embA = np.ascontiguousarray(embT[:, perm]).astype(ml_dtypes.bfloat16)
        embB = np.ascontiguousarray(embT[:, r0:r0 + R]).astype(ml_dtypes.bfloat16)

        # per-chunk window masks [128, MC, W]; window of chunk mc covers
        # rotated cols [128*mc, 128*mc + W)
        mask = np.zeros((128, MC, W), dtype=np.float32)
        for mc in range(MC):
            rows = slice(r0 + mc * 128, r0 + mc * 128 + 128)
            lab_rows = labs[rows]
            # class bounds of these rows must fall inside the window
            lo = int(starts[r0 + mc * 128]) - shift
            hi = int(ends[r0 + mc * 128 + 127]) - shift
            lo %= B
            hi = lo + ((hi - lo) % B)
            assert lo >= 128 * mc and hi <= 128 * mc + W, (
                f"core {c} chunk {mc}: class span [{lo},{hi}) outside "
                f"window [{128 * mc},{128 * mc + W})"
            )
            lab_win = labs[perm[128 * mc:128 * mc + W]]
            eq = lab_rows[:, None] == lab_win[None, :]
            mask[:, mc, :] = np.where(eq, np.float32(-4.0), np.float32(0.0))
        in_maps.append({
            "embA": embA,
            "embB": embB,
            "masks": mask.astype(ml_dtypes.float8_e4m3),
            "iden": iden,
        })
    return in_maps, valid


def _postprocess(results, valid):
    minv = np.zeros(B, dtype=np.float32)
    maxv = np.zeros(B, dtype=np.float32)
    for c, res in enumerate(results):
        # out [128, 10, MC]: 0=min, 1-4=exact max, 5-8=exp sums, 9=min2
        o = np.asarray(res["out"], np.float32)
        dmax = o[:, 1:5, :].max(axis=1)
        with np.errstate(divide="ignore"):
            smax = np.log(o[:, 5:9, :]).max(axis=1) / np.float32(T_SCALE)
        mx = np.maximum(dmax, smax)
        mn = np.minimum(o[:, 0, :], o[:, 9, :])
        for mc in range(MC):
            rows = slice(c * R + mc * 128, c * R + mc * 128 + 128)
            minv[rows] = mn[:, mc]
            maxv[rows] = mx[:, mc]
    hp = 1.0 - (minv + 4.0)   # hardest positive distance
    hn = 1.0 - maxv           # hardest negative distance
    per_row = np.maximum(0.0, hp - hn + MARGIN)
    cnt = int(valid.sum())
    if cnt == 0:
        return np.float32(0.0)
    return np.float32(np.sum(per_row[valid]) / cnt)


def run_device(in_maps, trace=False):
    from concourse.bass_utils import run_bass_kernel_spmd

    nc = _build_program()
    return run_bass_kernel_spmd(nc, in_maps, list(range(M)), trace=trace)


def kernel(emb, labels):
    in_maps, valid = _prep_inputs(emb, labels)
    out = run_device(in_maps, trace=False)
    return _postprocess(out.results, valid)


if __name__ == "__main__":
    # smoke test with random data
    rng = np.random.default_rng(0)
    emb = rng.standard_normal((B, D)).astype(np.float32)
    emb /= np.linalg.norm(emb, axis=1, keepdims=True) + 1e-12
    labels = rng.integers(0, 512, B).astype(np.int32)
    print(kernel(emb, labels))


# revision 15
# speedup vs baseline: 1.2165x; 1.0308x over previous
"""BatchHardTriplet loss kernel for Trainium2 (8 NeuronCores, SPMD).

Strategy
--------
Host sorts rows by label. Each core owns 1024 rows (8 chunks of 128). The
gathered operand embA is rotated per-core so that chunk mc's same-class
window lies in columns [128*mc, 128*mc+256) — provably sufficient while the
largest class has <= 64 members (host asserts). This makes the program
identical on all 8 cores (pure SPMD).

Per core, per chunk (128 rows x 8192 cols of the sim matrix):
  - PE: 16 matmuls of [128,512] bf16 fill 4 PSUM quads of [128,2048] fp32
    (2 physical quads, reused 2x per chunk). An identity @ mask_fp8 matmul
    accumulates -4 on same-label pairs inside the 256-col window, so they
    lose the global row-max (hardest negative) and win the window row-min
    (hardest positive, undone by +4 on host).
  - DVE: tensor_tensor_reduce drains two PSUM quads per chunk (2 fresh
    elems/cycle + free accumulation into per-chunk max slots) and a small
    256-col window min per chunk.
  - ACT: converts the other two quads to fp16 in SBUF.
  - GpSimd: pre-maxes the fp16 halves; a lagged DVE fp16 TTR finishes them.
Host applies validity and the final relu/mean exactly as the reference.
"""

import sys
import numpy as np

sys.path.insert(0, "/opt/trn_rl_repo")

B = 8192
D = 128
M = 8            # cores
R = B // M       # 1024 rows per core
MC = R // 128    # 8 chunks of 128 rows per core
W = 256          # per-chunk mask window width
NT = B // 512    # 16 column tiles of 512
MARGIN = 0.3
T_SCALE = 96.0   # logsumexp sharpness for ACT-accumulated quads

_CACHE = {}


def _build_program():
    """Build (once) the Bass program shared by all 8 cores."""
    if "nc" in _CACHE:
        return _CACHE["nc"]

    import concourse.bass as bass
    import concourse.bacc as bacc
    import concourse.mybir as mybir
    from concourse import tile

    f32 = mybir.dt.float32
    bf16 = mybir.dt.bfloat16
    fp16 = mybir.dt.float16
    fp8 = mybir.dt.float8e4
    Copy = mybir.ActivationFunctionType.Copy
    Exp = mybir.ActivationFunctionType.Exp
    AX = mybir.AxisListType.X
    MAX = mybir.AluOpType.max
    MIN = mybir.AluOpType.min

    nc = bacc.Bacc(None, target_bir_lowering=False)

    embA = nc.dram_tensor("embA", [D, B], fp8, kind="ExternalInput")
    embB = nc.dram_tensor("embB", [D, R], bf16, kind="ExternalInput")
    masks = nc.dram_tensor("masks", [128, MC, W], fp8, kind="ExternalInput")
    iden = nc.dram_tensor("iden", [128, 128], fp8, kind="ExternalInput")
    out = nc.dram_tensor("out", [128, 10, MC], f32, kind="ExternalOutput")

    with tile.TileContext(nc) as tc:
        with (
            tc.tile_pool(name="big", bufs=1) as big,
            tc.tile_pool(name="ps", bufs=4, space="PSUM") as ps,
            tc.tile_pool(name="epool", bufs=2) as epool,
            tc.tile_pool(name="st", bufs=1) as st,
        ):
            # DMA order: first-needed operands land first
            Bt = big.tile([D, R], bf16)
            nc.sync.dma_start(Bt[:], embB[:])
            A = [big.tile([D, 2048], fp8, name=f"A{s}") for s in range(4)]
            nc.sync.dma_start(A[0][:, 0:1024], embA[:, 0:1024])
            Id = big.tile([128, 128], fp8)
            nc.sync.dma_start(Id[:], iden[:])
            Mk = big.tile([128, MC, W], fp8)
            nc.sync.dma_start(Mk[:], masks[:])
            nc.sync.dma_start(A[0][:, 1024:2048], embA[:, 1024:2048])
            for s in range(1, 4):
                nc.sync.dma_start(A[s][:], embA[:, s * 2048:(s + 1) * 2048])

            O = st.tile([128, 10, MC], f32)
            nc.vector.memset(O[:, 9, :], 1.0e9)

            # warm up the PE activity monitor while the first DMAs land
            wp = ps.tile([128, 1024], f32, tag="psum", name="warm")
            for _ in range(6):
                nc.tensor.matmul(wp[:, 0:512], Bt[:, 0:128], Bt[:, 0:512],
                                 start=True, stop=True, skip_group_check=True)

            for mc in range(MC):
                lhsT = Bt[:, mc * 128:(mc + 1) * 128]
                wlo = 128 * mc          # window start col (cols 0..1151)
                whi = wlo + W

                H = []
                for h in range(8):
                    P = ps.tile([128, 1024], f32, tag="psum",
                                name=f"P{mc}_{h}")
                    H.append(P)
                    c0 = h * 1024
                    # does the window overlap this half?
                    for t in range(2):
                        lo = c0 + t * 512
                        wov = h < 2 and not (whi <= lo or wlo >= lo + 512)
                        nc.tensor.matmul(
                            P[:, t * 512:(t + 1) * 512], lhsT,
                            A[c0 // 2048][:, (c0 % 2048) + t * 512:
                                          (c0 % 2048) + (t + 1) * 512],
                            start=True, stop=not wov,
                        )
                    if h < 2:
                        # mask matmul segments inside this half
                        seg_lo = max(wlo, c0)
                        seg_hi = min(whi, c0 + 1024)
                        mm_lo = seg_lo
                        while mm_lo < seg_hi:
                            mm_hi = min(seg_hi, (mm_lo // 512 + 1) * 512)
                            nc.tensor.matmul(
                                P[:, mm_lo - c0:mm_hi - c0], Id[:],
                                Mk[:, mc, mm_lo - wlo:mm_hi - wlo],
                                start=False, stop=True,
                            )
                            mm_lo = mm_hi
                    # consumers: even halves -> ACT exp-accum, odd -> DVE max
                    if h % 2 == 0:
                        if h == 0:
                            nc.vector.tensor_reduce(
                                O[:, 0, mc:mc + 1],
                                P[:, wlo:min(whi, 1024)], axis=AX, op=MIN)
                        E = epool.tile([128, 1024], f32, tag="E",
                                       name=f"E{mc}_{h}")
                        nc.scalar.activation(
                            E[:], P[:], Exp, scale=T_SCALE,
                            accum_out=O[:, 5 + h // 2, mc:mc + 1])
                    else:
                        if h == 1 and whi > 1024:
                            nc.vector.tensor_reduce(
                                O[:, 9, mc:mc + 1],
                                P[:, 0:whi - 1024], axis=AX, op=MIN)
                        nc.vector.tensor_reduce(
                            O[:, 1 + h // 2, mc:mc + 1], P[:],
                            axis=AX, op=MAX)

            nc.sync.dma_start(out[:], O[:])

    nc.compile()
    _CACHE["nc"] = nc
    return nc


def _prep_inputs(emb, labels):
    """Sort by label, build per-core rotated operands + fp8 window masks."""
    import ml_dtypes

    emb = np.asarray(emb, dtype=np.float32)
    labels = np.asarray(labels)
    order = np.argsort(labels, kind="stable")
    labs = labels[order]
    embs = emb[order]
    embT = np.ascontiguousarray(embs.T)  # [D, B]

    starts = np.searchsorted(labs, labs, side="left")
    ends = np.searchsorted(labs, labs, side="right")
    counts = ends - starts
    valid = (counts >= 2) & (counts < B)

    iden = np.eye(128, dtype=ml_dtypes.float8_e4m3)

    in_maps = []
    for c in range(M):
        r0 = c * R
        shift = (int(starts[r0]) - 64) % B
        perm = (shift + np.arange(B)) % B
        embA = np.ascontiguousarray(embT[:, perm]).astype(ml_dtypes.float8_e4m3)
        embB = np.ascontiguousarray(embT[:, r0:r0 + R]).astype(ml_dtypes.bfloat16)

        # per-chunk window masks [128, MC, W]; window of chunk mc covers
        # rotated cols [128*mc, 128*mc + W)
        mask = np.zeros((128, MC, W), dtype=np.float32)
        for mc in range(MC):
            rows = slice(r0 + mc * 128, r0 + mc * 128 + 128)
            lab_rows = labs[rows]
            # class bounds of these rows must fall inside the window
            lo = int(starts[r0 + mc * 128]) - shift
            hi = int(ends[r0 + mc * 128 + 127]) - shift
            lo %= B
            hi = lo + ((hi - lo) % B)
            assert lo >= 128 * mc and hi <= 128 * mc + W, (
                f"core {c} chunk {mc}: class span [{lo},{hi}) outside "
                f"window [{128 * mc},{128 * mc + W})"
            )
            lab_win = labs[perm[128 * mc:128 * mc + W]]
            eq = lab_rows[:, None] == lab_win[None, :]
            mask[:, mc, :] = np.where(eq, np.float32(-4.0), np.float32(0.0))
        in_maps.append({
            "embA": embA,
            "embB": embB,
            "masks": mask.astype(ml_dtypes.float8_e4m3),
            "iden": iden,
        })
    return in_maps, valid


def _postprocess(results, valid):
    minv = np.zeros(B, dtype=np.float32)
    maxv = np.zeros(B, dtype=np.float32)
    for c, res in enumerate(results):
        # out [128, 10, MC]: 0=min, 1-4=exact max, 5-8=exp sums, 9=min2
        o = np.asarray(res["out"], np.float32)
        dmax = o[:, 1:5, :].max(axis=1)
        with np.errstate(divide="ignore"):
            smax = np.log(o[:, 5:9, :]).max(axis=1) / np.float32(T_SCALE)
        mx = np.maximum(dmax, smax)
        mn = np.minimum(o[:, 0, :], o[:, 9, :])
        for mc in range(MC):
            rows = slice(c * R + mc * 128, c * R + mc * 128 + 128)
            minv[rows] = mn[:, mc]
            maxv[rows] = mx[:, mc]
    hp = 1.0 - (minv + 4.0)   # hardest positive distance
    hn = 1.0 - maxv           # hardest negative distance
    per_row = np.maximum(0.0, hp - hn + MARGIN)
    cnt = int(valid.sum())
    if cnt == 0:
        return np.float32(0.0)
    return np.float32(np.sum(per_row[valid]) / cnt)


def run_device(in_maps, trace=False):
    from concourse.bass_utils import run_bass_kernel_spmd

    nc = _build_program()
    return run_bass_kernel_spmd(nc, in_maps, list(range(M)), trace=trace)


def kernel(emb, labels):
    in_maps, valid = _prep_inputs(emb, labels)
    out = run_device(in_maps, trace=False)
    return _postprocess(out.results, valid)


if __name__ == "__main__":
    # smoke test with random data
    rng = np.random.default_rng(0)
    emb = rng.standard_normal((B, D)).astype(np.float32)
    emb /= np.linalg.norm(emb, axis=1, keepdims=True) + 1e-12
    labels = rng.integers(0, 512, B).astype(np.int32)
    print(kernel(emb, labels))


# revision 18
# speedup vs baseline: 1.2658x; 1.0405x over previous
"""BatchHardTriplet loss kernel for Trainium2 (8 NeuronCores, SPMD).

Strategy
--------
Host sorts rows by label. Each core owns 1024 rows (8 chunks of 128). The
gathered operand embA is rotated per-core so that chunk mc's same-class
window lies in columns [128*mc, 128*mc+256) — provably sufficient while the
largest class has <= 64 members (host asserts). This makes the program
identical on all 8 cores (pure SPMD).

Per core, per chunk (128 rows x 8192 cols of the sim matrix):
  - PE: 16 matmuls of [128,512] bf16 fill 4 PSUM quads of [128,2048] fp32
    (2 physical quads, reused 2x per chunk). An identity @ mask_fp8 matmul
    accumulates -4 on same-label pairs inside the 256-col window, so they
    lose the global row-max (hardest negative) and win the window row-min
    (hardest positive, undone by +4 on host).
  - DVE: tensor_tensor_reduce drains two PSUM quads per chunk (2 fresh
    elems/cycle + free accumulation into per-chunk max slots) and a small
    256-col window min per chunk.
  - ACT: converts the other two quads to fp16 in SBUF.
  - GpSimd: pre-maxes the fp16 halves; a lagged DVE fp16 TTR finishes them.
Host applies validity and the final relu/mean exactly as the reference.
"""

import sys
import numpy as np

sys.path.insert(0, "/opt/trn_rl_repo")

B = 8192
D = 128
M = 8            # cores
R = B // M       # 1024 rows per core
MC = R // 128    # 8 chunks of 128 rows per core
W = 256          # per-chunk mask window width
NT = B // 512    # 16 column tiles of 512
MARGIN = 0.3
T_SCALE = 96.0   # logsumexp sharpness for ACT-accumulated quads

_CACHE = {}


def _build_program():
    """Build (once) the Bass program shared by all 8 cores."""
    if "nc" in _CACHE:
        return _CACHE["nc"]

    import concourse.bass as bass
    import concourse.bacc as bacc
    import concourse.mybir as mybir
    from concourse import tile

    f32 = mybir.dt.float32
    bf16 = mybir.dt.bfloat16
    fp16 = mybir.dt.float16
    fp8 = mybir.dt.float8e4
    Copy = mybir.ActivationFunctionType.Copy
    Exp = mybir.ActivationFunctionType.Exp
    AX = mybir.AxisListType.X
    MAX = mybir.AluOpType.max
    MIN = mybir.AluOpType.min

    nc = bacc.Bacc(None, target_bir_lowering=False)

    u8 = mybir.dt.uint8
    # blob1: embB bf16 (2048 B) | embA cols 0-2047 fp8 | Id fp8 | masks fp8
    blob1 = nc.dram_tensor("blob1", [128, 6272], u8, kind="ExternalInput")
    # blob2: embA cols 2048-8191 fp8
    blob2 = nc.dram_tensor("blob2", [128, 6144], fp8, kind="ExternalInput")
    out = nc.dram_tensor("out", [128, 10, MC], f32, kind="ExternalOutput")

    with tile.TileContext(nc) as tc:
        with (
            tc.tile_pool(name="big", bufs=1) as big,
            tc.tile_pool(name="ps", bufs=4, space="PSUM") as ps,
            tc.tile_pool(name="epool", bufs=2) as epool,
            tc.tile_pool(name="st", bufs=1) as st,
        ):
            # two packed input DMAs; first-needed bytes in blob1
            T1 = big.tile([128, 6272], u8, name="T1")
            nc.sync.dma_start(T1[:], blob1[:])
            T2 = big.tile([128, 6144], fp8, name="T2")
            nc.sync.dma_start(T2[:], blob2[:])
            T1b = T1.bitcast(bf16)    # [128, 3136]: cols 0-1023 = embB
            T18 = T1.bitcast(fp8)     # embA0 @2048, Id @4096, masks @4224
            wrm = big.tile([128, 512], bf16, name="wrm")
            nc.vector.memset(wrm[:], 0.0)

            O = st.tile([128, 10, MC], f32)
            nc.vector.memset(O[:, 9, :], 1.0e9)

            # warm up the PE activity monitor while the input DMAs land
            wp = ps.tile([128, 1024], f32, tag="psum", name="warm")
            for _ in range(8):
                nc.tensor.matmul(wp[:, 0:512], wrm[:, 0:128], wrm[:],
                                 start=True, stop=True, skip_group_check=True)

            for mc in range(MC):
                lhsT = T1b[:, mc * 128:(mc + 1) * 128]
                wlo = 128 * mc          # window start col (cols 0..1151)
                whi = wlo + W

                H = []
                for h in range(8):
                    P = ps.tile([128, 1024], f32, tag="psum",
                                name=f"P{mc}_{h}")
                    H.append(P)
                    c0 = h * 1024
                    # does the window overlap this half?
                    for t in range(2):
                        lo = c0 + t * 512
                        wov = h < 2 and not (whi <= lo or wlo >= lo + 512)
                        src_lo = c0 + t * 512
                        if src_lo < 2048:
                            rhs = T18[:, 2048 + src_lo:2048 + src_lo + 512]
                        else:
                            rhs = T2[:, src_lo - 2048:src_lo - 2048 + 512]
                        nc.tensor.matmul(
                            P[:, t * 512:(t + 1) * 512], lhsT, rhs,
                            start=True, stop=not wov,
                        )
                    if h < 2:
                        # mask matmul segments inside this half
                        seg_lo = max(wlo, c0)
                        seg_hi = min(whi, c0 + 1024)
                        mm_lo = seg_lo
                        while mm_lo < seg_hi:
                            mm_hi = min(seg_hi, (mm_lo // 512 + 1) * 512)
                            nc.tensor.matmul(
                                P[:, mm_lo - c0:mm_hi - c0],
                                T18[:, 4096:4224],
                                T18[:, 4224 + mc * W + mm_lo - wlo:
                                     4224 + mc * W + mm_hi - wlo],
                                start=False, stop=True,
                            )
                            mm_lo = mm_hi
                    # consumers: even halves -> ACT exp-accum, odd -> DVE max
                    if h % 2 == 0:
                        if h == 0:
                            nc.vector.tensor_reduce(
                                O[:, 0, mc:mc + 1],
                                P[:, wlo:min(whi, 1024)], axis=AX, op=MIN)
                        E = epool.tile([128, 1024], f32, tag="E",
                                       name=f"E{mc}_{h}")
                        nc.scalar.activation(
                            E[:], P[:], Exp, scale=T_SCALE,
                            accum_out=O[:, 5 + h // 2, mc:mc + 1])
                    else:
                        if h == 1 and whi > 1024:
                            nc.vector.tensor_reduce(
                                O[:, 9, mc:mc + 1],
                                P[:, 0:whi - 1024], axis=AX, op=MIN)
                        nc.vector.tensor_reduce(
                            O[:, 1 + h // 2, mc:mc + 1], P[:],
                            axis=AX, op=MAX)

            nc.sync.dma_start(out[:], O[:])

    nc.compile()
    _CACHE["nc"] = nc
    return nc


def _prep_inputs(emb, labels):
    """Sort by label, build per-core rotated operands + fp8 window masks."""
    import ml_dtypes

    emb = np.asarray(emb, dtype=np.float32)
    labels = np.asarray(labels)
    order = np.argsort(labels, kind="stable")
    labs = labels[order]
    embs = emb[order]
    embT = np.ascontiguousarray(embs.T)  # [D, B]

    starts = np.searchsorted(labs, labs, side="left")
    ends = np.searchsorted(labs, labs, side="right")
    counts = ends - starts
    valid = (counts >= 2) & (counts < B)

    iden = np.eye(128, dtype=ml_dtypes.float8_e4m3)

    in_maps = []
    for c in range(M):
        r0 = c * R
        shift = (int(starts[r0]) - 64) % B
        perm = (shift + np.arange(B)) % B
        embA = np.ascontiguousarray(embT[:, perm]).astype(ml_dtypes.float8_e4m3)
        embB = np.ascontiguousarray(embT[:, r0:r0 + R]).astype(ml_dtypes.bfloat16)

        # per-chunk window masks [128, MC, W]; window of chunk mc covers
        # rotated cols [128*mc, 128*mc + W)
        mask = np.zeros((128, MC, W), dtype=np.float32)
        for mc in range(MC):
            rows = slice(r0 + mc * 128, r0 + mc * 128 + 128)
            lab_rows = labs[rows]
            # class bounds of these rows must fall inside the window
            lo = int(starts[r0 + mc * 128]) - shift
            hi = int(ends[r0 + mc * 128 + 127]) - shift
            lo %= B
            hi = lo + ((hi - lo) % B)
            assert lo >= 128 * mc and hi <= 128 * mc + W, (
                f"core {c} chunk {mc}: class span [{lo},{hi}) outside "
                f"window [{128 * mc},{128 * mc + W})"
            )
            lab_win = labs[perm[128 * mc:128 * mc + W]]
            eq = lab_rows[:, None] == lab_win[None, :]
            mask[:, mc, :] = np.where(eq, np.float32(-4.0), np.float32(0.0))
        mask8 = mask.astype(ml_dtypes.float8_e4m3)
        b1 = np.empty((128, 6272), dtype=np.uint8)
        b1[:, 0:2048] = np.ascontiguousarray(embB).view(np.uint8)
        b1[:, 2048:4096] = embA[:, 0:2048].copy().view(np.uint8)
        b1[:, 4096:4224] = iden.view(np.uint8)
        b1[:, 4224:6272] = mask8.reshape(128, MC * W).view(np.uint8)
        b2 = np.ascontiguousarray(embA[:, 2048:8192])
        in_maps.append({"blob1": b1, "blob2": b2})
    return in_maps, valid


def _postprocess(results, valid):
    minv = np.zeros(B, dtype=np.float32)
    maxv = np.zeros(B, dtype=np.float32)
    for c, res in enumerate(results):
        # out [128, 10, MC]: 0=min, 1-4=exact max, 5-8=exp sums, 9=min2
        o = np.asarray(res["out"], np.float32)
        dmax = o[:, 1:5, :].max(axis=1)
        with np.errstate(divide="ignore"):
            smax = np.log(o[:, 5:9, :]).max(axis=1) / np.float32(T_SCALE)
        mx = np.maximum(dmax, smax)
        mn = np.minimum(o[:, 0, :], o[:, 9, :])
        for mc in range(MC):
            rows = slice(c * R + mc * 128, c * R + mc * 128 + 128)
            minv[rows] = mn[:, mc]
            maxv[rows] = mx[:, mc]
    hp = 1.0 - (minv + 4.0)   # hardest positive distance
    hn = 1.0 - maxv           # hardest negative distance
    per_row = np.maximum(0.0, hp - hn + MARGIN)
    cnt = int(valid.sum())
    if cnt == 0:
        return np.float32(0.0)
    return np.float32(np.sum(per_row[valid]) / cnt)


def run_device(in_maps, trace=False):
    from concourse.bass_utils import run_bass_kernel_spmd

    nc = _build_program()
    return run_bass_kernel_spmd(nc, in_maps, list(range(M)), trace=trace)


def kernel(emb, labels):
    in_maps, valid = _prep_inputs(emb, labels)
    out = run_device(in_maps, trace=False)
    return _postprocess(out.results, valid)


if __name__ == "__main__":
    # smoke test with random data
    rng = np.random.default_rng(0)
    emb = rng.standard_normal((B, D)).astype(np.float32)
    emb /= np.linalg.norm(emb, axis=1, keepdims=True) + 1e-12
    labels = rng.integers(0, 512, B).astype(np.int32)
    print(kernel(emb, labels))
